# revision 1
# baseline (speedup 1.0000x reference)
# MoE (8 experts, top-2) on 8 TRN2 NeuronCores — expert-parallel.
#
# Host (numpy): router matmul + softmax + top-2 (exactly mirrors the jax
# reference arithmetic in fp32), then dispatch: gather each expert's tokens
# into a [D, C] column block (bf16, pre-transposed for the device matmul
# layout), C = max expert load (exact, no tile padding).
# Device (per core, expert e): hT = gelu_tanh(W1[e]^T @ xT + b1), then
# yT = (W2[e]^T @ hT) * gate — both matmuls bf16 with fp32 PSUM
# accumulation. mm2 is computed transposed (tokens on the moving/free axis)
# so BOTH matmuls scale with the exact token count instead of quantizing to
# 128-token tiles; the gate broadcast multiplies along the free axis on DVE,
# fused with the PSUM evacuation.
# Host: transpose each expert's [D, n_e] result and scatter-add into [N, D].
#
# Shapes are hardcoded for B=4, S=2048, D=1024, H=4096, E=8 (spec), but the
# builder is parametric in the per-expert capacity C (known only after
# routing), so the Bass program is built after routing on every call.

import numpy as np
import ml_dtypes

NUM_EXPERTS = 8
TOP_K = 2
P = 128          # SBUF partitions
TB = 512         # token block (matmul moving free size)

_program_cache = {}


def _build_program(C, D, H):
    import concourse.mybir as mybir
    import concourse.tile as tile
    from concourse import bacc

    bf = mybir.dt.bfloat16
    f32 = mybir.dt.float32
    Gelu = mybir.ActivationFunctionType.Gelu_apprx_tanh

    KD = D // P      # contraction chunks for mm1 / output row chunks (8)
    KH = H // P      # contraction chunks for mm2 (32)

    MJ = 16
    HJ = H // MJ

    # All inputs arrive pre-packed by the host in SBUF layout (partition dim
    # first, load-unit contiguous) so every DMA streams contiguous runs per
    # partition at full bandwidth:
    #   xt : [P, KD*C]   block-packed: block b occupies [KD*t0, KD*(t0+tbs))
    #   w1 : [P, MJ, KD, HJ]  column-slice-major
    #   w2 : [P, KH, D]
    nc = bacc.Bacc(None, target_bir_lowering=False, debug=False)
    KD_ = D // P
    xt = nc.declare_dram_parameter("xt", [P, KD_ * C], bf, isOutput=False).ap()
    w1 = nc.declare_dram_parameter("w1", [P, MJ, KD_, HJ], bf, isOutput=False).ap()
    w2 = nc.declare_dram_parameter("w2", [P, H // P, D], bf, isOutput=False).ap()
    gb = nc.declare_dram_parameter("gb", [P, C], f32, isOutput=False).ap()
    b1t = nc.declare_dram_parameter("b1t", [P, H // P], f32, isOutput=False).ap()
    ytr = nc.declare_dram_parameter("ytr", [D, C], f32, isOutput=True).ap()

    # remainder block LAST: block 0 must be full-size so its ~110 us of
    # compute covers the W2 + next-block streams (a short first block
    # exposes a ~20 us PE stall waiting on W2)
    rem = C % TB
    sizes = [TB] * (C // TB) + ([rem] if rem else [])

    with tile.TileContext(nc) as tc:
        with (
            tc.tile_pool(name="weights", bufs=1) as wpool,
            tc.tile_pool(name="xin", bufs=2) as xpool,
            tc.tile_pool(name="hbuf", bufs=1) as hpool,
            tc.tile_pool(name="yout", bufs=3) as ypool,
            tc.tile_pool(name="gates", bufs=2) as gbp,
            tc.tile_pool(name="ph", bufs=5, space="PSUM") as php,
            tc.tile_pool(name="py", bufs=3, space="PSUM") as pyp,
        ):
            # Resident weights. W1 lives as MJ column-slice tiles (each holds
            # all KD contraction chunks for a range of 4 output m-tiles) so
            # the first matmul group only waits for ~1 MiB of W1, and later
            # slices stream in behind the compute. W2 is emitted after
            # block 0's activations in the same DMA queue (needed ~55 us in).
            w1_sb = [
                wpool.tile([P, KD, HJ], bf, tag=f"w1sb{j}", name=f"w1sb{j}")
                for j in range(MJ)
            ]
            w2_sb = wpool.tile([P, KH, D], bf, tag="w2sb")
            b1_sb = wpool.tile([P, H // P], f32, tag="b1sb")

            nc.sync.dma_start(b1_sb, b1t)
            nc.sync.dma_start(w1_sb[0], w1[:, 0, :, :])

            t0 = 0
            for b, tbs in enumerate(sizes):
                xt_blk = xpool.tile([P, KD, tbs], bf, tag="xt")
                nc.sync.dma_start(
                    xt_blk,
                    xt[:, KD * t0:KD * (t0 + tbs)].rearrange(
                        "p (k c) -> p k c", k=KD
                    ),
                )
                if b == 0:
                    for j in range(1, MJ):
                        nc.sync.dma_start(w1_sb[j], w1[:, j, :, :])
                # gate broadcast is only needed at mm2, so it queues after
                # the W1 stream (keeps the startup-critical window clear)
                gb_sb = gbp.tile([P, tbs], f32, tag="gb")
                nc.sync.dma_start(gb_sb, gb[:, t0:t0 + tbs])
                if b == 0:
                    nc.sync.dma_start(w2_sb, w2)
                # mm1: hT[m] = gelu(W1_chunk^T @ xT_block + b1) -> [P, tbs]
                hT = hpool.tile([P, KH, tbs], bf, tag="hT")
                for m in range(KH):
                    ph = php.tile([P, tbs], f32, tag="ph")
                    mj, mo = divmod(m, HJ // P)
                    for k in range(KD):
                        nc.tensor.matmul(
                            ph,
                            w1_sb[mj][:, k, mo * P:(mo + 1) * P],
                            xt_blk[:, k, :],
                            start=(k == 0),
                            stop=(k == KD - 1),
                        )
                    nc.scalar.activation(
                        hT[:, m, :], ph, Gelu, bias=b1_sb[:, m:m + 1]
                    )
                # mm2 (transposed): yT[d] = (W2_chunk^T @ hT_block) * gate
                for d in range(KD):
                    pyT = pyp.tile([P, tbs], f32, tag="py")
                    for k in range(KH):
                        nc.tensor.matmul(
                            pyT,
                            w2_sb[:, k, d * P:(d + 1) * P],
                            hT[:, k, :],
                            start=(k == 0),
                            stop=(k == KH - 1),
                        )
                    # fused PSUM evacuation + gate broadcast multiply on DVE
                    # (keeps ACT exclusively on Gelu so its LUT stays warm)
                    yt = ypool.tile([P, tbs], f32, tag="yt")
                    nc.vector.tensor_mul(yt, pyT, gb_sb)
                    nc.sync.dma_start(ytr[d * P:(d + 1) * P, t0:t0 + tbs], yt)
                t0 += tbs
    nc.compile()
    return nc


def _ensure_trace_hooks():
    # bass_utils' trace path (taken when BASS_TRACE=1 is set externally)
    # imports antenv.axon_hooks, which this image lacks. Shim it (and the
    # artifact upload, which needs a bucket) only when missing, so tracing
    # degrades gracefully instead of crashing.
    import sys
    import types

    try:
        import antenv.axon_hooks  # noqa: F401
        return
    except ImportError:
        pass
    try:
        import antenv

        mod = types.ModuleType("antenv.axon_hooks")
        state = {"hook": None}
        mod.set_axon_ntff_profile_hook = lambda h: state.__setitem__("hook", h)
        mod.get_axon_ntff_profile_hook = lambda: state["hook"]
        sys.modules["antenv.axon_hooks"] = mod
        antenv.axon_hooks = mod
        try:
            from trn_agent_boot.trn_boot import _ntff_profile_via_ctypes

            mod.set_axon_ntff_profile_hook(
                _ntff_profile_via_ctypes("/opt/axon/libaxon_pjrt.so")
            )
            import concourse.bass_utils as _bu

            _orig_upload = _bu.upload_artifacts

            def _safe_upload(tmpdir):
                try:
                    return _orig_upload(tmpdir)
                except Exception:
                    return f"local:{tmpdir}"

            _bu.upload_artifacts = _safe_upload
        except Exception:
            pass
    except Exception:
        pass


def kernel(x, Wr, W1, b1, W2, b2):
    _ensure_trace_hooks()
    from concourse.bass_utils import run_bass_kernel_spmd

    bf16 = ml_dtypes.bfloat16
    B, S, D = x.shape
    E, _, H = W1.shape
    N = B * S
    xm = np.ascontiguousarray(x.reshape(N, D), dtype=np.float32)

    # --- host router (mirrors reference fp32 arithmetic; softmax is
    # monotonic so top-k on probs == top-k on logits, ties broken by index)
    logits = xm @ Wr
    mx = logits.max(axis=1, keepdims=True)
    ex = np.exp(logits - mx)
    probs = ex / ex.sum(axis=1, keepdims=True)
    top_i = np.argsort(-probs, axis=1, kind="stable")[:, :TOP_K]

    idx = [np.where((top_i == e).any(axis=1))[0] for e in range(E)]
    counts = np.array([len(i) for i in idx])
    C = max(P, int(counts.max()))  # exact capacity, no tile padding

    # --- dispatch: pack everything in SBUF layout (partition-major,
    # load-unit contiguous) so device DMAs stream at full bandwidth
    KD = D // P
    MJ = 16
    HJ = H // MJ
    rem = C % TB
    sizes = [TB] * (C // TB) + ([rem] if rem else [])  # must match builder
    xT = np.ascontiguousarray(xm.T).astype(bf16)  # [D, N]
    in_maps = []
    for e in range(E):
        xte = np.zeros((D, C), dtype=bf16)
        xte[:, :counts[e]] = xT[:, idx[e]]
        xte3 = xte.reshape(KD, P, C).transpose(1, 0, 2)  # [P, KD, C]
        t0 = 0
        chunks = []
        for tbs in sizes:
            chunks.append(xte3[:, :, t0:t0 + tbs].reshape(P, -1))
            t0 += tbs
        xtp = np.ascontiguousarray(np.concatenate(chunks, axis=1))  # [P, KD*C]
        ge = np.zeros((C,), dtype=np.float32)
        ge[:counts[e]] = probs[idx[e], e]
        w1b = np.asarray(W1[e], dtype=np.float32).astype(bf16)
        w2b = np.asarray(W2[e], dtype=np.float32).astype(bf16)
        in_maps.append({
            "xt": xtp,
            "w1": np.ascontiguousarray(
                w1b.reshape(KD, P, MJ, HJ).transpose(1, 2, 0, 3)
            ),
            "w2": np.ascontiguousarray(
                w2b.reshape(H // P, P, D).transpose(1, 0, 2)
            ),
            "gb": np.ascontiguousarray(np.broadcast_to(ge, (P, C))),
            "b1t": np.ascontiguousarray(
                np.asarray(b1[e], dtype=np.float32).reshape(H // P, P).T
            ),
        })

    key = (C, D, H)
    if key not in _program_cache:
        _program_cache[key] = _build_program(C, D, H)
    nc = _program_cache[key]

    res = run_bass_kernel_spmd(nc, in_maps, core_ids=list(range(NUM_EXPERTS)))

    # --- combine: transpose each expert's [D, n_e] block and scatter-add
    # (indices unique per expert)
    out = np.zeros((N, D), dtype=np.float32)
    b2f = np.asarray(b2, dtype=np.float32)
    for e in range(E):
        ytr = np.asarray(res.results[e]["ytr"])
        ye = np.ascontiguousarray(ytr[:, :counts[e]].T, dtype=np.float32)
        if b2f[e].any():
            ye = ye + probs[idx[e], e][:, None] * b2f[e]
        out[idx[e]] += ye
    return out.reshape(B, S, D)

